# revision 25
# baseline (speedup 1.0000x reference)
"""Trainium2 Bass kernel for nn_CRF: 5 mean-field iterations of
y = x + w*blur(softmax(.)) on (16,384,384,21) f32, data-parallel over batch
across 8 NeuronCores (2 samples per core).

Self-contained: builds the Bass program, shards inputs, runs via
bass_utils.run_bass_kernel_spmd, reassembles the full output.

Device formulation (per sample, free layout (c,w) per H-row partition):
    e0 = exp(x) (host, bf16), p = softmax(x) (host, bf16, iter-0 input)
    per iter: z = Bh-conv(p)  [transpose-style matmul, H-contraction]
              s = Bw-conv(z)  [transpose-style matmul, W-contraction]
              e = exp(s) * e0 ; p = e / sum_c(e)
    final iter: y = s (f32 add of x happens host-side)

Engine budget (cost-model): Act is pinned by exp (only exp engine); the
PSUM->SBUF f32->bf16 z-cast has no fast path anywhere (0.833ns/elem on
Pool/Act); all bf16 SBUF elementwise runs on DVE at the 2x_1p rate
(0.521ns/elem). Work is split across Act/DVE/Pool by the *_PCT knobs.
"""
import sys
if "/opt/trn_rl_repo" not in sys.path:
    sys.path.insert(0, "/opt/trn_rl_repo")

import os as _os
import numpy as np
import ml_dtypes

H = W = 384
C = 21
FREE = C * W  # 8064
FS = 9
HALF = FS // 2  # 4
N_ITER = 5
SAMPLES_PER_CORE = 2
N_CORES = 8

# K-piece table: (src_tile, k0, k1, n0, n1)
# src partitions [k0 .. k1) of tile, output band columns [n0, n1).
# Within each overlap region the wide (pending->overwrite) piece must come
# before the narrow accumulating one, so every matmul's written region is
# homogeneous for PSUM per-element accumulate semantics.
PIECES = [
    (0, 0, 128, 0, 124),      # t0 main
    (1, 0, 128, 124, 244),    # t1 main
    (0, 0, 128, 124, 132),    # t0->t1 boundary (8 cols, rows 120-127 nonzero)
    (2, 0, 128, 244, 364),    # t2 main (cols 244-251 are zero rows -> writes 0)
    (1, 0, 128, 244, 260),    # t1 tail: [244,252) alone + [252,260) overlap with t2
    (2, 0, 128, 364, 384),    # t2 tail (20 cols)
]
NPAD = 124  # padded band columns in DRAM

GSIZE = int(_os.environ.get("CRF_G", "2"))  # channels per PSUM group
PSUM_BUFS = int(_os.environ.get("CRF_PSUM_BUFS", "4"))
CGROUPS = [(c, min(c + GSIZE, C)) for c in range(0, C, GSIZE)]
NG = len(CGROUPS)

# engine-split knobs (percent thresholds, deterministic per-chunk hash)
ZACT_N = int(_os.environ.get("CRF_ZACT_N", "6"))       # first N casts/iter on Act
ZDVE_PCT = int(_os.environ.get("CRF_ZDVE", "0"))       # % of remaining casts on DVE
SUMPOOL = _os.environ.get("CRF_SUMPOOL", "010")        # per-i: 1 -> sum chain on Pool
PMULPOOL_PCT = int(_os.environ.get("CRF_PMULPOOL", "45"))  # % of p-muls on Pool
EMULPOOL_PCT = int(_os.environ.get("CRF_EMULPOOL", "0"))   # % of e-muls on Pool


def gauss_taps(inv_theta, spacing):
    d = spacing * np.arange(-HALF, HALF + 1, dtype=np.float64)
    k = np.exp(-((d * inv_theta) ** 2) / 2.0)
    k[HALF] = 0.0
    return k


def band_pieces(taps):
    """[6, 128, NPAD] f32 band matrices at absolute partition rows [k0,k1):
    band[p][k0+k, n] = taps[(k_abs - n_abs) + HALF]."""
    out = np.zeros((len(PIECES), 128, NPAD), dtype=np.float64)
    for p, (t, k0, k1, n0, n1) in enumerate(PIECES):
        k_abs = t * 128 + np.arange(k0, k1)
        n_abs = np.arange(n0, n1)
        d = k_abs[:, None] - n_abs[None, :] + HALF
        m = (d >= 0) & (d < FS)
        out[p, k0:k1, : n1 - n0] = np.where(m, taps[np.clip(d, 0, FS - 1)], 0.0)
    return out.astype(np.float32)


def prep_inputs(x, spatial_spacings, smoothness_weight, inv_smoothness_theta):
    """Full inputs -> list of 8 per-core input dicts (host-side prep)."""
    x = np.asarray(x, dtype=np.float32)
    sp = np.asarray(spatial_spacings, dtype=np.float32)
    wgt = float(np.asarray(smoothness_weight))
    ith = np.asarray(inv_smoothness_theta, dtype=np.float32)

    B = x.shape[0]
    xt = np.ascontiguousarray(x.transpose(0, 1, 3, 2)).reshape(B, H, FREE)  # (B,H,(C,W))
    ef = np.exp(xt.reshape(B, H, C, W))
    s0 = ef.sum(axis=2, keepdims=True)
    p0 = (ef / s0).reshape(B, H, FREE).astype(ml_dtypes.bfloat16)
    e0 = ef.reshape(B, H, FREE).astype(ml_dtypes.bfloat16)

    xb = xt.astype(ml_dtypes.bfloat16)
    ident = np.eye(128, dtype=ml_dtypes.bfloat16)
    in_maps = []
    prep_inputs.last_xt = xt  # stashed for unpack_outputs host-side add
    for core in range(N_CORES):
        bs = [core * SAMPLES_PER_CORE + i for i in range(SAMPLES_PER_CORE)]
        bh = np.stack([band_pieces(gauss_taps(ith[0], sp[b, 0])) for b in bs])
        bw = np.stack(
            [band_pieces(gauss_taps(ith[1], sp[b, 1])) * wgt for b in bs]
        )
        in_maps.append(
            {
                "p0": np.ascontiguousarray(p0[bs]),
                "xb": np.ascontiguousarray(xb[bs]),
                "ident": ident,
                "bh": bh.astype(ml_dtypes.bfloat16),
                "bw": bw.astype(ml_dtypes.bfloat16),
            }
        )
    return in_maps


def unpack_outputs(results, xt=None):
    """list of per-core {'y': [2,H,FREE] bf16 s-values} -> full (16,H,W,C) f32.
    The final y = x + s add happens here on host in f32."""
    if xt is None:
        xt = prep_inputs.last_xt
    ss = np.concatenate([np.asarray(r["y"], dtype=np.float32) for r in results],
                        axis=0)  # (B, H, FREE)
    ys = xt[:ss.shape[0]] + ss
    return np.ascontiguousarray(
        ys.reshape(-1, H, C, W).transpose(0, 1, 3, 2)
    )  # (B,H,W,C)


def build_program(num_devices=N_CORES):
    import concourse.bacc as bacc
    import concourse.mybir as mybir
    import concourse.tile as tile

    f32 = mybir.dt.float32
    bf16 = mybir.dt.bfloat16
    AF = mybir.ActivationFunctionType

    nc = bacc.Bacc("TRN2", target_bir_lowering=False, debug=False,
                   num_devices=num_devices)

    S = SAMPLES_PER_CORE
    p0_d = nc.dram_tensor("p0", [S, H, FREE], bf16, kind="ExternalInput")
    xb_d = nc.dram_tensor("xb", [S, H, FREE], bf16, kind="ExternalInput")
    id_d = nc.dram_tensor("ident", [128, 128], bf16, kind="ExternalInput")
    bh_d = nc.dram_tensor("bh", [S, 6, 128, NPAD], bf16, kind="ExternalInput")
    bw_d = nc.dram_tensor("bw", [S, 6, 128, NPAD], bf16, kind="ExternalInput")
    y_d = nc.dram_tensor("y", [S, H, FREE], bf16, kind="ExternalOutput")

    def pick(k, pct):
        return (k * 37 + 11) % 100 < pct

    with tile.TileContext(nc) as tc:
        with (
            tc.tile_pool(name="res", bufs=1) as res,      # big residents
            tc.tile_pool(name="small", bufs=1) as small,  # bands, sums, rb
            tc.tile_pool(name="chunk", bufs=6) as chunk,  # y staging
            tc.tile_pool(name="psum1", bufs=PSUM_BUFS, space="PSUM") as psum1,
        ):
            for b in range(S):
                # --- residents for this sample (tags shared across samples:
                # samples run sequentially through the same buffers) ---
                e_t = [res.tile([128, FREE], bf16, name=f"e{i}_s{b}", tag=f"e{i}")
                       for i in range(3)]
                z_t = [res.tile([128, FREE], bf16, name=f"z{i}_s{b}", tag=f"z{i}")
                       for i in range(3)]
                x_t = [res.tile([128, FREE], bf16, name=f"x{i}_s{b}", tag=f"x{i}")
                       for i in range(3)]
                id_t = small.tile([128, 128], bf16, name=f"id_s{b}", tag="id")
                bh_t = [small.tile([128, NPAD], bf16, name=f"bh{p}_s{b}", tag=f"bh{p}")
                        for p in range(6)]
                bw_t = [small.tile([128, NPAD], bf16, name=f"bw{p}_s{b}", tag=f"bw{p}")
                        for p in range(6)]
                sm_t = [small.tile([128, W], bf16, name=f"sm{i}_s{b}", tag=f"sm{i}")
                        for i in range(3)]
                rb_t = [small.tile([128, W], bf16, name=f"rb{i}_s{b}", tag=f"rb{i}")
                        for i in range(3)]

                for p in range(6):
                    nc.sync.dma_start(bh_t[p][:], bh_d[b, p])
                    nc.sync.dma_start(bw_t[p][:], bw_d[b, p])
                nc.sync.dma_start(id_t[:], id_d[0:128, 0:128])
                ldq = [nc.sync, nc.scalar, nc.sync]
                for i in range(3):
                    ldq[i].dma_start(e_t[i][:], p0_d[b, 128 * i:128 * (i + 1), :])
                for i in range(3):
                    nc.gpsimd.dma_start(x_t[i][:], xb_d[b, 128 * i:128 * (i + 1), :])

                def s1_tile(it, gi, j):
                    """H-conv matmuls + z-cast for channel group gi, w-chunk j."""
                    c0, c1 = CGROUPS[gi]
                    G = c1 - c0
                    ps = psum1.tile([128, GSIZE * 512], f32,
                                    name=f"ps1_{b}_{it}_{j}_{c0}", tag="ps")
                    for ci, c in enumerate(range(c0, c1)):
                        for p, (t, k0, k1, n0, n1) in enumerate(PIECES):
                            nc.tensor.matmul(
                                ps[:, ci * 512 + n0: ci * 512 + n1],
                                e_t[t][k0:k1, c * W + 128 * j: c * W + 128 * (j + 1)],
                                bh_t[p][k0:k1, 0:n1 - n0],
                                start=(p == 0),
                                stop=(p == len(PIECES) - 1),
                            )
                    zdst = z_t[j][:, c0 * W: c1 * W].rearrange(
                        "p (c n) -> p c n", c=G)
                    zsrc = ps.rearrange("p (c n) -> p c n", c=GSIZE)[:, 0:G, 0:W]
                    # The first ZACT_N casts of each iteration run during the
                    # previous iteration's softmax window, when Act is idle;
                    # they also pre-feed the next exp stream.
                    ordn = gi * 3 + j
                    if ordn < ZACT_N:
                        nc.scalar.copy(zdst, zsrc)
                    elif (ordn * 37) % 100 < ZDVE_PCT:
                        nc.vector.tensor_copy(zdst, zsrc)
                    else:
                        nc.gpsimd.tensor_copy(zdst, zsrc)

                def s2_group(it, i, gi, last):
                    """W-conv matmuls for (h-tile i, group gi) + exp*e0 or y."""
                    c0, c1 = CGROUPS[gi]
                    G = c1 - c0
                    ps = psum1.tile([128, GSIZE * 512], f32,
                                    name=f"ps2_{b}_{it}_{i}_{c0}", tag="ps")
                    ps3 = ps.rearrange("p (c n) -> p c n", c=GSIZE)[:, 0:G, 0:W]
                    for ci, c in enumerate(range(c0, c1)):
                        for p, (t, k0, k1, n0, n1) in enumerate(PIECES):
                            nc.tensor.matmul(
                                ps[:, ci * 512 + n0: ci * 512 + n1],
                                z_t[t][k0:k1, c * W + 128 * i: c * W + 128 * (i + 1)],
                                bw_t[p][k0:k1, 0:n1 - n0],
                                start=(p == 0),
                                stop=last and (p == len(PIECES) - 1),
                            )
                    esl = e_t[i][:, c0 * W: c1 * W]
                    if not last:
                        # e = exp(s + x): x is folded into PSUM by identity
                        # matmuls on PE (start=False accumulate), saving the
                        # es*e0 elementwise multiply entirely. One matmul per
                        # channel: a matmul output may not cross a PSUM bank.
                        for ci, c in enumerate(range(c0, c1)):
                            nc.tensor.matmul(
                                ps[:, ci * 512: ci * 512 + W],
                                id_t[:],
                                x_t[i][:, c * W:(c + 1) * W],
                                start=False, stop=True)
                        nc.scalar.activation(
                            esl.rearrange("p (c n) -> p c n", c=G), ps3, AF.Exp)
                        # incremental channel-sum: sm_i accumulates during the
                        # pipelined phase, so no bulk reduction tail remains
                        e3 = e_t[i].rearrange("p (c w) -> p c w", c=C)
                        seng = nc.gpsimd if SUMPOOL[i] == "1" else nc.vector
                        with nc.allow_low_precision("bf16 softmax sums"):
                            if gi == 0:
                                seng.tensor_add(sm_t[i][:], e3[:, 0, :],
                                                e3[:, 1, :])
                            else:
                                for c in range(c0, c1):
                                    seng.tensor_add(sm_t[i][:], sm_t[i][:],
                                                    e3[:, c, :])
                    else:
                        yo = chunk.tile([128, GSIZE * W], bf16,
                                        name=f"yo_{b}_{it}_{i}_{c0}", tag="yo")
                        ydst = yo[:, 0:G * W].rearrange("p (c n) -> p c n", c=G)
                        k = i * NG + gi
                        yeng = [nc.scalar, nc.vector, nc.gpsimd][k % 3]
                        if yeng is nc.scalar:
                            yeng.copy(ydst, ps3)
                        else:
                            yeng.tensor_copy(ydst, ps3)
                        nc.gpsimd.dma_start(
                            y_d[b, 128 * i:128 * (i + 1), c0 * W: c1 * W],
                            yo[:, 0:G * W])

                def recip_i(it, i):
                    rb = rb_t[i]
                    with nc.allow_low_precision("1/sumexp in bf16"):
                        nc.vector.reciprocal(rb[:], sm_t[i][:])

                def pmul_ij(it, i, j):
                    """p = e * rb for h-tile i, w-chunk j (chunked by j so next
                    iteration's stage-1 (j) can start early)."""
                    e3 = e_t[i].rearrange("p (c w) -> p c w", c=C)
                    wsl = slice(128 * j, 128 * (j + 1))
                    peng = (nc.gpsimd if pick(i * 3 + j + 77, PMULPOOL_PCT)
                            else nc.vector)
                    peng.tensor_mul(
                        e3[:, :, wsl], e3[:, :, wsl],
                        rb_t[i][:, wsl].unsqueeze(1).to_broadcast([128, C, 128]),
                    )

                for it in range(N_ITER):
                    last = it == N_ITER - 1
                    # Software-pipelined: slot g emits stage-1 group g
                    # interleaved tile-by-tile with stage-2 groups of g-1, so
                    # consecutive PSUM consumers land on different engines
                    # (cast on Pool, exp on Act) and overlap through the
                    # 2-buffer PSUM FIFO.
                    for gs in range(NG):
                        for k in range(3):
                            s1_tile(it, gs, k)
                            if gs >= 1:
                                s2_group(it, k, gs - 1, last)
                    for i in range(3):
                        s2_group(it, i, NG - 1, last)
                    if not last:
                        for i in range(3):
                            recip_i(it, i)
                        # j-major: the j=0 chunks finish first across all i,
                        # unblocking next iteration's stage-1 w-chunk 0 early
                        for j in range(3):
                            for i in range(3):
                                pmul_ij(it, i, j)

    nc.compile()
    return nc


def kernel(x, spatial_spacings, smoothness_weight, inv_smoothness_theta):
    import sys
    if "/opt/trn_rl_repo" not in sys.path:
        sys.path.insert(0, "/opt/trn_rl_repo")
    from concourse.bass_utils import run_bass_kernel_spmd

    in_maps = prep_inputs(x, spatial_spacings, smoothness_weight,
                          inv_smoothness_theta)
    nc = build_program()
    res = run_bass_kernel_spmd(nc, in_maps, core_ids=list(range(N_CORES)))
    return unpack_outputs(res.results)


# revision 30
# speedup vs baseline: 1.0419x; 1.0419x over previous
"""Trainium2 Bass kernel for nn_CRF: 5 mean-field iterations of
y = x + w*blur(softmax(.)) on (16,384,384,21) f32, data-parallel over batch
across 8 NeuronCores (2 samples per core).

Self-contained: builds the Bass program, shards inputs, runs via
bass_utils.run_bass_kernel_spmd, reassembles the full output.

Device formulation (per sample, free layout (c,w) per H-row partition):
    e0 = exp(x) (host, bf16), p = softmax(x) (host, bf16, iter-0 input)
    per iter: z = Bh-conv(p)  [transpose-style matmul, H-contraction]
              s = Bw-conv(z)  [transpose-style matmul, W-contraction]
              e = exp(s) * e0 ; p = e / sum_c(e)
    final iter: y = s (f32 add of x happens host-side)

Engine budget (cost-model): Act is pinned by exp (only exp engine); the
PSUM->SBUF f32->bf16 z-cast has no fast path anywhere (0.833ns/elem on
Pool/Act); all bf16 SBUF elementwise runs on DVE at the 2x_1p rate
(0.521ns/elem). Work is split across Act/DVE/Pool by the *_PCT knobs.
"""
import sys
if "/opt/trn_rl_repo" not in sys.path:
    sys.path.insert(0, "/opt/trn_rl_repo")

import os as _os
import numpy as np
import ml_dtypes

H = W = 384
C = 21
FREE = C * W  # 8064
FS = 9
HALF = FS // 2  # 4
N_ITER = 5
SAMPLES_PER_CORE = 2
N_CORES = 8

# K-piece table: (src_tile, k0, k1, n0, n1)
# src partitions [k0 .. k1) of tile, output band columns [n0, n1).
# Within each overlap region the wide (pending->overwrite) piece must come
# before the narrow accumulating one, so every matmul's written region is
# homogeneous for PSUM per-element accumulate semantics.
PIECES = [
    (0, 0, 128, 0, 124),      # t0 main
    (1, 0, 128, 124, 244),    # t1 main
    (0, 0, 128, 124, 132),    # t0->t1 boundary (8 cols, rows 120-127 nonzero)
    (2, 0, 128, 244, 364),    # t2 main (cols 244-251 are zero rows -> writes 0)
    (1, 0, 128, 244, 260),    # t1 tail: [244,252) alone + [252,260) overlap with t2
    (2, 0, 128, 364, 384),    # t2 tail (20 cols)
]
NPAD = 124  # padded band columns in DRAM

GSIZE = int(_os.environ.get("CRF_G", "2"))  # channels per PSUM group
PSUM_BUFS = int(_os.environ.get("CRF_PSUM_BUFS", "4"))
CGROUPS = [(c, min(c + GSIZE, C)) for c in range(0, C, GSIZE)]
NG = len(CGROUPS)

# engine-split knobs (percent thresholds, deterministic per-chunk hash)
ZACT_N = int(_os.environ.get("CRF_ZACT_N", "6"))       # first N casts/iter on Act
ZDVE_PCT = int(_os.environ.get("CRF_ZDVE", "0"))       # % of remaining casts on DVE
SUMPOOL = _os.environ.get("CRF_SUMPOOL", "010")        # per-i: 1 -> sum chain on Pool
PMULPOOL_PCT = int(_os.environ.get("CRF_PMULPOOL", "45"))  # % of p-muls on Pool
EMULPOOL_PCT = int(_os.environ.get("CRF_EMULPOOL", "0"))   # % of e-muls on Pool


def gauss_taps(inv_theta, spacing):
    d = spacing * np.arange(-HALF, HALF + 1, dtype=np.float64)
    k = np.exp(-((d * inv_theta) ** 2) / 2.0)
    k[HALF] = 0.0
    return k


def band_pieces(taps):
    """[6, 128, NPAD] f32 band matrices at absolute partition rows [k0,k1):
    band[p][k0+k, n] = taps[(k_abs - n_abs) + HALF]."""
    out = np.zeros((len(PIECES), 128, NPAD), dtype=np.float64)
    for p, (t, k0, k1, n0, n1) in enumerate(PIECES):
        k_abs = t * 128 + np.arange(k0, k1)
        n_abs = np.arange(n0, n1)
        d = k_abs[:, None] - n_abs[None, :] + HALF
        m = (d >= 0) & (d < FS)
        out[p, k0:k1, : n1 - n0] = np.where(m, taps[np.clip(d, 0, FS - 1)], 0.0)
    return out.astype(np.float32)


def prep_inputs(x, spatial_spacings, smoothness_weight, inv_smoothness_theta):
    """Full inputs -> list of 8 per-core input dicts (host-side prep)."""
    x = np.asarray(x, dtype=np.float32)
    sp = np.asarray(spatial_spacings, dtype=np.float32)
    wgt = float(np.asarray(smoothness_weight))
    ith = np.asarray(inv_smoothness_theta, dtype=np.float32)

    B = x.shape[0]
    xt = np.ascontiguousarray(x.transpose(0, 1, 3, 2)).reshape(B, H, FREE)  # (B,H,(C,W))
    ef = np.exp(xt.reshape(B, H, C, W))
    s0 = ef.sum(axis=2, keepdims=True)
    p0 = (ef / s0).reshape(B, H, FREE).astype(ml_dtypes.bfloat16)
    e0 = ef.reshape(B, H, FREE).astype(ml_dtypes.bfloat16)

    in_maps = []
    prep_inputs.last_xt = xt  # stashed for unpack_outputs host-side add
    for core in range(N_CORES):
        bs = [core * SAMPLES_PER_CORE + i for i in range(SAMPLES_PER_CORE)]
        bh = np.stack([band_pieces(gauss_taps(ith[0], sp[b, 0])) for b in bs])
        bw = np.stack(
            [band_pieces(gauss_taps(ith[1], sp[b, 1])) * wgt for b in bs]
        )
        in_maps.append(
            {
                "p0": np.ascontiguousarray(p0[bs]),
                "e0": np.ascontiguousarray(e0[bs]),
                "bh": bh.astype(ml_dtypes.bfloat16),
                "bw": bw.astype(ml_dtypes.bfloat16),
            }
        )
    return in_maps


def unpack_outputs(results, xt=None):
    """list of per-core {'y': [2,H,FREE] bf16 s-values} -> full (16,H,W,C) f32.
    The final y = x + s add happens here on host in f32."""
    if xt is None:
        xt = prep_inputs.last_xt
    ss = np.concatenate([np.asarray(r["y"], dtype=np.float32) for r in results],
                        axis=0)  # (B, H, FREE)
    ys = xt[:ss.shape[0]] + ss
    return np.ascontiguousarray(
        ys.reshape(-1, H, C, W).transpose(0, 1, 3, 2)
    )  # (B,H,W,C)


def build_program(num_devices=N_CORES):
    import concourse.bacc as bacc
    import concourse.mybir as mybir
    import concourse.tile as tile

    f32 = mybir.dt.float32
    bf16 = mybir.dt.bfloat16
    AF = mybir.ActivationFunctionType

    nc = bacc.Bacc("TRN2", target_bir_lowering=False, debug=False,
                   num_devices=num_devices)

    S = SAMPLES_PER_CORE
    p0_d = nc.dram_tensor("p0", [S, H, FREE], bf16, kind="ExternalInput")
    e0_d = nc.dram_tensor("e0", [S, H, FREE], bf16, kind="ExternalInput")
    bh_d = nc.dram_tensor("bh", [S, 6, 128, NPAD], bf16, kind="ExternalInput")
    bw_d = nc.dram_tensor("bw", [S, 6, 128, NPAD], bf16, kind="ExternalInput")
    y_d = nc.dram_tensor("y", [S, H, FREE], bf16, kind="ExternalOutput")

    def pick(k, pct):
        return (k * 37 + 11) % 100 < pct

    with tile.TileContext(nc) as tc:
        with (
            tc.tile_pool(name="res", bufs=1) as res,      # big residents
            tc.tile_pool(name="small", bufs=1) as small,  # bands, sums, rb
            tc.tile_pool(name="chunk", bufs=6) as chunk,  # y staging
            tc.tile_pool(name="psum1", bufs=PSUM_BUFS, space="PSUM") as psum1,
        ):
            for b in range(S):
                # --- residents for this sample (tags shared across samples:
                # samples run sequentially through the same buffers) ---
                e_t = [res.tile([128, FREE], bf16, name=f"e{i}_s{b}", tag=f"e{i}")
                       for i in range(3)]
                z_t = [res.tile([128, FREE], bf16, name=f"z{i}_s{b}", tag=f"z{i}")
                       for i in range(3)]
                e0_t = [res.tile([128, FREE], bf16, name=f"e0{i}_s{b}", tag=f"e0{i}")
                        for i in range(3)]
                bh_t = [small.tile([128, NPAD], bf16, name=f"bh{p}_s{b}", tag=f"bh{p}")
                        for p in range(6)]
                bw_t = [small.tile([128, NPAD], bf16, name=f"bw{p}_s{b}", tag=f"bw{p}")
                        for p in range(6)]
                sm_t = [small.tile([128, W], bf16, name=f"sm{i}_s{b}", tag=f"sm{i}")
                        for i in range(3)]
                rb_t = [small.tile([128, W], bf16, name=f"rb{i}_s{b}", tag=f"rb{i}")
                        for i in range(3)]

                for p in range(6):
                    nc.sync.dma_start(bh_t[p][:], bh_d[b, p])
                    nc.sync.dma_start(bw_t[p][:], bw_d[b, p])
                ldq = [nc.sync, nc.scalar, nc.sync]
                for i in range(3):
                    ldq[i].dma_start(e_t[i][:], p0_d[b, 128 * i:128 * (i + 1), :])
                for i in range(3):
                    nc.gpsimd.dma_start(e0_t[i][:], e0_d[b, 128 * i:128 * (i + 1), :])

                def s1_tile(it, gi, j):
                    """H-conv matmuls + z-cast for channel group gi, w-chunk j."""
                    c0, c1 = CGROUPS[gi]
                    G = c1 - c0
                    ps = psum1.tile([128, GSIZE * 512], f32,
                                    name=f"ps1_{b}_{it}_{j}_{c0}", tag="ps")
                    for ci, c in enumerate(range(c0, c1)):
                        for p, (t, k0, k1, n0, n1) in enumerate(PIECES):
                            nc.tensor.matmul(
                                ps[:, ci * 512 + n0: ci * 512 + n1],
                                e_t[t][k0:k1, c * W + 128 * j: c * W + 128 * (j + 1)],
                                bh_t[p][k0:k1, 0:n1 - n0],
                                start=(p == 0),
                                stop=(p == len(PIECES) - 1),
                            )
                    zdst = z_t[j][:, c0 * W: c1 * W].rearrange(
                        "p (c n) -> p c n", c=G)
                    zsrc = ps.rearrange("p (c n) -> p c n", c=GSIZE)[:, 0:G, 0:W]
                    # The first ZACT_N casts of each iteration run during the
                    # previous iteration's softmax window, when Act is idle;
                    # they also pre-feed the next exp stream.
                    ordn = gi * 3 + j
                    if ordn < ZACT_N:
                        nc.scalar.copy(zdst, zsrc)
                    elif (ordn * 37) % 100 < ZDVE_PCT:
                        nc.vector.tensor_copy(zdst, zsrc)
                    else:
                        nc.gpsimd.tensor_copy(zdst, zsrc)

                def s2_group(it, i, gi, last):
                    """W-conv matmuls for (h-tile i, group gi) + exp*e0 or y."""
                    c0, c1 = CGROUPS[gi]
                    G = c1 - c0
                    ps = psum1.tile([128, GSIZE * 512], f32,
                                    name=f"ps2_{b}_{it}_{i}_{c0}", tag="ps")
                    ps3 = ps.rearrange("p (c n) -> p c n", c=GSIZE)[:, 0:G, 0:W]
                    for ci, c in enumerate(range(c0, c1)):
                        for p, (t, k0, k1, n0, n1) in enumerate(PIECES):
                            nc.tensor.matmul(
                                ps[:, ci * 512 + n0: ci * 512 + n1],
                                z_t[t][k0:k1, c * W + 128 * i: c * W + 128 * (i + 1)],
                                bw_t[p][k0:k1, 0:n1 - n0],
                                start=(p == 0),
                                stop=(p == len(PIECES) - 1),
                            )
                    esl = e_t[i][:, c0 * W: c1 * W]
                    if not last:
                        # e_t[i] region: exp(s) in place, then *= e0
                        nc.scalar.activation(
                            esl.rearrange("p (c n) -> p c n", c=G), ps3, AF.Exp)
                        meng = (nc.gpsimd if pick(i * NG + gi, EMULPOOL_PCT)
                                else nc.vector)
                        meng.tensor_mul(esl, esl, e0_t[i][:, c0 * W: c1 * W])
                        # incremental channel-sum: sm_i accumulates during the
                        # pipelined phase, so no bulk reduction tail remains
                        e3 = e_t[i].rearrange("p (c w) -> p c w", c=C)
                        seng = nc.gpsimd if SUMPOOL[i] == "1" else nc.vector
                        with nc.allow_low_precision("bf16 softmax sums"):
                            if gi == 0:
                                seng.tensor_add(sm_t[i][:], e3[:, 0, :],
                                                e3[:, 1, :])
                            else:
                                for c in range(c0, c1):
                                    seng.tensor_add(sm_t[i][:], sm_t[i][:],
                                                    e3[:, c, :])
                    else:
                        yo = chunk.tile([128, GSIZE * W], bf16,
                                        name=f"yo_{b}_{it}_{i}_{c0}", tag="yo")
                        ydst = yo[:, 0:G * W].rearrange("p (c n) -> p c n", c=G)
                        k = i * NG + gi
                        yeng = [nc.scalar, nc.vector, nc.gpsimd][k % 3]
                        if yeng is nc.scalar:
                            yeng.copy(ydst, ps3)
                        else:
                            yeng.tensor_copy(ydst, ps3)
                        nc.gpsimd.dma_start(
                            y_d[b, 128 * i:128 * (i + 1), c0 * W: c1 * W],
                            yo[:, 0:G * W])

                def recip_i(it, i):
                    rb = rb_t[i]
                    with nc.allow_low_precision("1/sumexp in bf16"):
                        nc.vector.reciprocal(rb[:], sm_t[i][:])

                def pmul_ij(it, i, j):
                    """p = e * rb for h-tile i, w-chunk j (chunked by j so next
                    iteration's stage-1 (j) can start early)."""
                    e3 = e_t[i].rearrange("p (c w) -> p c w", c=C)
                    wsl = slice(128 * j, 128 * (j + 1))
                    peng = (nc.gpsimd if pick(i * 3 + j + 77, PMULPOOL_PCT)
                            else nc.vector)
                    peng.tensor_mul(
                        e3[:, :, wsl], e3[:, :, wsl],
                        rb_t[i][:, wsl].unsqueeze(1).to_broadcast([128, C, 128]),
                    )

                for it in range(N_ITER):
                    last = it == N_ITER - 1
                    # Software-pipelined: slot g emits stage-1 group g
                    # interleaved tile-by-tile with stage-2 groups of g-1, so
                    # consecutive PSUM consumers land on different engines
                    # (cast on Pool, exp on Act) and overlap through the
                    # 2-buffer PSUM FIFO.
                    for gs in range(NG):
                        for k in range(3):
                            s1_tile(it, gs, k)
                            if gs >= 1:
                                s2_group(it, k, gs - 1, last)
                    for i in range(3):
                        s2_group(it, i, NG - 1, last)
                    if not last:
                        for i in range(3):
                            recip_i(it, i)
                        # j-major: the j=0 chunks finish first across all i,
                        # unblocking next iteration's stage-1 w-chunk 0 early
                        for j in range(3):
                            for i in range(3):
                                pmul_ij(it, i, j)

    nc.compile()
    return nc


def kernel(x, spatial_spacings, smoothness_weight, inv_smoothness_theta):
    import sys
    if "/opt/trn_rl_repo" not in sys.path:
        sys.path.insert(0, "/opt/trn_rl_repo")
    from concourse.bass_utils import run_bass_kernel_spmd

    in_maps = prep_inputs(x, spatial_spacings, smoothness_weight,
                          inv_smoothness_theta)
    nc = build_program()
    res = run_bass_kernel_spmd(nc, in_maps, core_ids=list(range(N_CORES)))
    return unpack_outputs(res.results)
